# revision 1
# baseline (speedup 1.0000x reference)
"""Trainium2 Bass kernel: segment-mean over token segments + pairwise-diff edge MLP.

Reference computation (per batch row b):
  seg = cumsum(ids == 3); valid = ids != 3
  means[n] = mean of features[s] over tokens with seg==n & valid (n < 8), 0-count -> sum/1
  diff[i,j] = means[i] - means[j]                          # [8,8,H]
  out[i,j]  = relu(relu(diff @ W1 + b1) @ Wm + bm) @ W2 + b2   # [8,8,150]

Distribution: data-parallel over batch B=128 across 8 NeuronCores (16 rows/core),
tiny MLP weights replicated, no cross-core communication.

The kernel is HBM/TensorE co-bound. Features are cast to bf16 on the host
(tolerance is 2e-2; bf16 rounding contributes ~4e-3), halving HBM traffic and
doubling PE stream rate vs fp32, and laid out so each SBUF partition line is
contiguous in DRAM (token s -> partition s//8, chunk s%8 => 12 KB descriptors).

Schedule notes (from perfetto traces):
  - feature rows stream on the sync HWDGE queue only; row 0 is split into
    four chunk-quarters so the first matmul can start ~7us earlier, and rows
    4..15 ride in 2-row batched DMAs to amortize the ~1.2us per-dma overhead.
  - 3-stage software pipeline: stage1(g) / diff(g-1) / mm1-mm3(g-2). TensorE
    is in-order, so deferring each MLP stage gives its cross-engine inputs
    (means evictions, diff casts) a full stage-1 group of slack and keeps the
    matmul stream from stalling and dropping the PE p-state (the PE ramps
    0.65 -> 1.2 -> 2.4 GHz over ~10us of continuous work; dummy matmuls spin
    it hot through the startup window).
  - engine separation: stage-1 PSUM evictions live on the scalar queue only;
    the whole MLP post-matmul path (casts, fused bias+relu tensor_scalar, b2
    adds) lives on vector only - the two pipelines never queue behind each
    other.
  - bulk MLP weights ride the slow gpsimd SWDGE path - latency is hidden by
    the deferred MLP; ohT+biases go early on the scalar queue; mid-stream
    output DMAs use gpsimd, the final latency-critical pair scalar+sync.

Device algorithm per core:
  stage1: means [8seg, 768] per row via TensorE: onehot*(1/count) stationary
          (host-precomputed, bf16) x features (moving, bf16) accumulated over
          8 token chunks; plain cast eviction PSUM->SBUF (scalar/vector split).
  diff:   diffT = means^T @ E4 where E4 is a constant +-1 selection matrix ->
          fuses the transpose AND the pairwise difference. Stage-1 stacks 2
          rows per PSUM bank at partition stripes 0/32, so diff needs only
          two K=128 matmuls per h-chunk (block-diagonal E4, zero gap rows).
          Output columns = (g2, b2, i, j) = 256 per 4 rows.
  MLP:    transposed matmuls, contraction dim on partitions, c-dim split 128+22.
          Biases b1/bm applied as per-partition activation bias (c on
          partitions); b2 added by DVE during the final eviction. Output is
          [rows, 150] fp32.
"""

import sys

import numpy as np
import ml_dtypes

if "/opt/trn_rl_repo" not in sys.path:
    sys.path.insert(0, "/opt/trn_rl_repo")

import concourse.bass as bass
import concourse.mybir as mybir
from concourse.bass import ds
from concourse.bass_utils import run_bass_kernel_spmd
from concourse.tile import TileContext

B, S, H, C = 128, 1024, 768, 150
NSEG = 8
SEP_ID = 3
NCORES = 8
RPC = B // NCORES  # 16 rows per core
TCH = S // 128     # 8 token chunks
HC = H // 128      # 6 hidden chunks
HHALF = 384        # H split for PSUM bank limit
CC = ((0, 128), (128, 22))  # c-dim (150) chunks

F32 = mybir.dt.float32
BF16 = mybir.dt.bfloat16
NPBF16 = ml_dtypes.bfloat16


def build_program(rpc=RPC, tch=TCH):
    ngp = rpc // 4  # groups of 4 batch rows -> 256 output rows each
    nc = bass.Bass("TRN2", target_bir_lowering=False, debug=False)

    feats_d = nc.dram_tensor("features", [rpc, tch * 128, H], BF16,
                             kind="ExternalInput").ap()
    ohT_d = nc.dram_tensor("ohT", [128, rpc * tch * NSEG], BF16,
                           kind="ExternalInput").ap()
    w1p_d = nc.dram_tensor("w1p", [128, HC * C], BF16, kind="ExternalInput").ap()
    wm0_d = nc.dram_tensor("wm0", [128, C], BF16, kind="ExternalInput").ap()
    wm1_d = nc.dram_tensor("wm1", [22, C], BF16, kind="ExternalInput").ap()
    w20_d = nc.dram_tensor("w20", [128, C], BF16, kind="ExternalInput").ap()
    w21_d = nc.dram_tensor("w21", [22, C], BF16, kind="ExternalInput").ap()
    b1c0_d = nc.dram_tensor("b1c0", [128, 1], F32, kind="ExternalInput").ap()
    b1c1_d = nc.dram_tensor("b1c1", [22, 1], F32, kind="ExternalInput").ap()
    bm0_d = nc.dram_tensor("bm0", [128, 1], F32, kind="ExternalInput").ap()
    bm1_d = nc.dram_tensor("bm1", [22, 1], F32, kind="ExternalInput").ap()
    b2bc_d = nc.dram_tensor("b2bc", [128, C], F32, kind="ExternalInput").ap()
    # striped block-diagonal E4, one 256-col block per 2-row pair:
    # partition (r4%2)*32+n, column (pair, g2, b2, i, j)
    e4_d = nc.dram_tensor("e4", [128, 512], BF16, kind="ExternalInput").ap()
    out_d = nc.dram_tensor("out", [ngp * 256, C], F32, kind="ExternalOutput").ap()

    RELU = mybir.ActivationFunctionType.Relu
    COPY = mybir.ActivationFunctionType.Copy

    with TileContext(nc) as tc:
        with (
            tc.tile_pool(name="const", bufs=1) as constp,
            tc.tile_pool(name="f0p", bufs=1) as f0p,
            tc.tile_pool(name="f1p", bufs=1) as f1p,
            tc.tile_pool(name="f23p", bufs=2) as f23p,
            tc.tile_pool(name="fpairp", bufs=5) as fpairp,
            tc.tile_pool(name="meansp", bufs=4) as meansp,
            tc.tile_pool(name="diffp", bufs=2) as diffp,
            tc.tile_pool(name="actp", bufs=2) as actp,
            tc.tile_pool(name="osbp", bufs=3) as osbp,
            tc.tile_pool(name="mpsum", bufs=4, space="PSUM") as mpsum,
            tc.tile_pool(name="dpsum", bufs=2, space="PSUM") as dpsum,
            tc.tile_pool(name="hpsum", bufs=2, space="PSUM") as hpsum,
        ):
            # scalar queue: ohT + small consts, early and fast
            ohT_sb = constp.tile([128, rpc * tch * NSEG], BF16, tag="c_ohT")
            nc.scalar.dma_start(out=ohT_sb, in_=ohT_d)
            b1_sb = []
            for ci, (coff, csz) in enumerate(CC):
                t = constp.tile([csz, 1], F32, tag=f"c_b1_{ci}")
                nc.scalar.dma_start(out=t, in_=(b1c0_d, b1c1_d)[ci])
                b1_sb.append(t)
            bm_sb = []
            for ci, (coff, csz) in enumerate(CC):
                t = constp.tile([csz, 1], F32, tag=f"c_bm_{ci}")
                nc.scalar.dma_start(out=t, in_=(bm0_d, bm1_d)[ci])
                bm_sb.append(t)
            b2bc_sb = constp.tile([128, C], F32, tag="c_b2bc")
            nc.scalar.dma_start(out=b2bc_sb, in_=b2bc_d)
            # gpsimd SWDGE: bulk MLP weights (latency hidden by deferred MLP)
            e4_sb = constp.tile([128, 512], BF16, tag="c_e4")
            nc.gpsimd.dma_start(out=e4_sb, in_=e4_d)
            w1_sb = constp.tile([128, HC * C], BF16, tag="c_w1")
            nc.gpsimd.dma_start(out=w1_sb, in_=w1p_d)
            wm0_sb = constp.tile([128, C], BF16, tag="c_wm0")
            nc.gpsimd.dma_start(out=wm0_sb, in_=wm0_d)
            wm1_sb = constp.tile([22, C], BF16, tag="c_wm1")
            nc.gpsimd.dma_start(out=wm1_sb, in_=wm1_d)
            w20_sb = constp.tile([128, C], BF16, tag="c_w20")
            nc.gpsimd.dma_start(out=w20_sb, in_=w20_d)
            w21_sb = constp.tile([22, C], BF16, tag="c_w21")
            nc.gpsimd.dma_start(out=w21_sb, in_=w21_d)

            # PE p-state warmup: the PE ramps 0.65 -> 1.2 -> 2.4 GHz over
            # ~10us of continuous work. Spin dummy matmuls through the ~11us
            # feature-DMA startup window so row 0 lands on a hot array.
            # Stage-1 PSUM banks hold 4 rows' [8, HHALF] stripes at partition
            # bases 0/32/64/96 (the only bases matmul tile_position allows).
            # Zero all 4 slots once so the never-written gap partitions stay
            # finite zeros forever (only stripes are written after this) -
            # the diff matmul contracts over all 128 partitions and relies on
            # gap x 0 = 0.
            dmy = constp.tile([128, HHALF], BF16, tag="c_dmy")
            nc.vector.memset(dmy, 0.0)
            wmps = []
            for i in range(4):
                wt = mpsum.tile([128, HHALF], F32, tag="mp", name=f"z{i}")
                # full-width zero matmul writes all 128 partitions of the
                # bank (0 x 0 accumulation with start=True)
                nc.tensor.matmul(wt, dmy[:, 0:128], dmy,
                                 start=True, stop=True)
                wmps.append(wt)
            for i in range(22):
                nc.tensor.matmul(wmps[i % 2][ds(0, NSEG), :],
                                 dmy[:, 0:NSEG], dmy,
                                 start=True, stop=True)

            feat_view = {}  # row -> fn(t) -> [128, H] AP for chunk t

            def dma_feat_rows(row, nrows, pool, tag):
                """One dma_start covering `nrows` rows starting at `row`."""
                ft = pool.tile([128, nrows, tch, H], BF16, tag=tag)
                nc.sync.dma_start(
                    out=ft,
                    in_=feats_d[ds(row, nrows)].rearrange(
                        "r (p t) h -> p r t h", t=tch),
                )
                for k in range(nrows):
                    feat_view[row + k] = (
                        lambda t, ft=ft, k=k: ft[:, k, t, :])

            def dma_feat_chunks(row, pool, tag, tgrp):
                """Row split into len(tgrp) DMAs by token-chunk ranges."""
                fr = feats_d[row].rearrange("(p t) h -> p t h", t=tch)
                tiles = {}
                for lo, hi in tgrp:
                    ft = pool.tile([128, hi - lo, H], BF16, tag=f"{tag}_{lo}")
                    nc.sync.dma_start(out=ft, in_=fr[:, ds(lo, hi - lo), :])
                    for t in range(lo, hi):
                        tiles[t] = (ft, t - lo)
                feat_view[row] = (
                    lambda t, tiles=tiles: tiles[t][0][:, tiles[t][1], :])

            def stage1(gp):
                """Segment means for the 4 rows of group gp: 2 rows per PSUM
                bank pair at partition stripes 0/32 (the only extra base the
                hw allows is 64; 96 is rejected). Each pair is evicted as
                soon as its 2 rows finish, so bank reuse never stalls the
                next group. Gap partitions stay zero from the startup
                bank-zeroing matmuls."""
                ms = []
                for pair in range(2):
                    mph = [mpsum.tile([128, HHALF], F32, tag="mp",
                                      name=f"mp{gp}_{pair}_{h}")
                           for h in range(2)]
                    m = meansp.tile([128, H], BF16, tag="means")
                    for sr in range(2):
                        r4 = pair * 2 + sr
                        row = gp * 4 + r4
                        if row == 0:
                            dma_feat_chunks(row, f0p, "f0",
                                            [(0, 2), (2, 4), (4, 6), (6, 8)])
                        elif row == 1:
                            dma_feat_chunks(row, f1p, "f1",
                                            [(0, 4), (4, 8)])
                        elif row in (2, 3):
                            dma_feat_rows(row, 1, f23p, "f23")
                        elif row >= 4 and row % 2 == 0:
                            dma_feat_rows(row, 2, fpairp, "fpair")
                        fv = feat_view[row]
                        for half in range(2):
                            for t in range(tch):
                                nc.tensor.matmul(
                                    mph[half][ds(sr * 32, NSEG), :],
                                    ohT_sb[:, ds((row * tch + t) * NSEG,
                                                 NSEG)],
                                    fv(t)[:, ds(half * HHALF, HHALF)],
                                    start=(t == 0),
                                    stop=(t == tch - 1),
                                )
                    for half in range(2):
                        # scalar-only: stage-1 evictions never share a queue
                        # with the MLP post-matmul path (all on vector)
                        nc.scalar.activation(
                            m[:, ds(half * HHALF, HHALF)], mph[half], COPY)
                    ms.append(m)
                return ms

            def mlp_front(gp, ms):
                """Pairwise diff (fused transpose): diffT = means^T @ E4 as
                two accumulating K=128 matmuls per h-chunk over the striped
                means pairs (block-diagonal E4 has zero rows at the gaps).
                Runs one gp behind stage1."""
                diff = diffp.tile([128, HC, 256], BF16, tag="diff")
                for hp2 in range(HC // 2):
                    # two h-chunks share one 2KB bank (column ranges 0-255 /
                    # 256-511, sequential accumulation groups) -> one cast
                    dp = dpsum.tile([128, 512], F32, tag="dp")
                    for k in range(2):
                        hc = 2 * hp2 + k
                        for pair in range(2):
                            nc.tensor.matmul(
                                dp[:, ds(k * 256, 256)],
                                ms[pair][:, ds(hc * 128, 128)],
                                e4_sb[:, ds(pair * 256, 256)],
                                start=(pair == 0), stop=(pair == 1))
                    nc.vector.tensor_copy(
                        diff[:, ds(2 * hp2, 2), :],
                        dp.rearrange("p (a b) -> p a b", a=2))
                return diff

            def mlp_back(gp, diff, last=False):
                """MLP body; runs two gps behind stage1 so the diff casts it
                consumes have a full stage1 group of slack."""
                # ---- mm1: h1T = relu(W1^T @ diffT + b1) ----
                h1 = []
                for ci, (coff, csz) in enumerate(CC):
                    hp = hpsum.tile([csz, 256], F32, tag="hp")
                    for hc in range(HC):
                        nc.tensor.matmul(
                            hp,
                            w1_sb[:, ds(hc * C + coff, csz)],
                            diff[:, hc, :],
                            start=(hc == 0),
                            stop=(hc == HC - 1),
                        )
                    hs = actp.tile([csz, 256], BF16, tag=f"h1s{ci}")
                    if ci == 1:
                        # parallel with ci=0 on vector: mm2's ldweights waits
                        # on both, and the tensor stream is in-order
                        nc.scalar.activation(hs, hp, RELU, bias=b1_sb[ci])
                    else:
                        nc.vector.tensor_scalar(hs, hp, b1_sb[ci], 0.0,
                                                mybir.AluOpType.add,
                                                mybir.AluOpType.max)
                    h1.append(hs)

                # ---- mm2: h2T = relu(Wm^T @ h1T + bm) ----
                h2 = []
                for ci, (coff, csz) in enumerate(CC):
                    hp = hpsum.tile([csz, 256], F32, tag="hp")
                    nc.tensor.matmul(hp, wm0_sb[:, ds(coff, csz)],
                                     h1[0], start=True, stop=False)
                    nc.tensor.matmul(hp, wm1_sb[:, ds(coff, csz)],
                                     h1[1], start=False, stop=True)
                    hs = actp.tile([csz, 256], BF16, tag=f"h2s{ci}")
                    if ci == 1:
                        nc.scalar.activation(hs, hp, RELU, bias=bm_sb[ci])
                    else:
                        nc.vector.tensor_scalar(hs, hp, bm_sb[ci], 0.0,
                                                mybir.AluOpType.add,
                                                mybir.AluOpType.max)
                    h2.append(hs)

                # ---- mm3: out = h2 @ W2 + b2, natural [rows, c] layout ----
                for rs in range(2):
                    op = dpsum.tile([128, 512], F32, tag="dp")
                    nc.tensor.matmul(op[:, 0:C], h2[0][:, ds(rs * 128, 128)],
                                     w20_sb, start=True, stop=False)
                    nc.tensor.matmul(op[:, 0:C], h2[1][:, ds(rs * 128, 128)],
                                     w21_sb, start=False, stop=True)
                    osb = osbp.tile([128, C], F32, tag="osb")
                    nc.vector.tensor_add(osb, op[:, 0:C], b2bc_sb)
                    # mid-stream outs ride the idle gpsimd queue; the final
                    # (latency-critical) pair overlaps on scalar + the
                    # now-drained sync queue
                    if last:
                        eng = nc.scalar if rs == 0 else nc.sync
                    else:
                        eng = nc.gpsimd
                    eng.dma_start(
                        out=out_d[ds(gp * 256 + rs * 128, 128), :], in_=osb
                    )

            # back(g) is emitted BEFORE front(g+1): when back's mm1/mm2
            # complete, the vector queue is empty of fresh diff casts, so the
            # latency-critical activations start immediately. Both stages
            # keep a full group of slack on their inputs.
            means_of, diff_of = {}, {}
            for gp in range(ngp):
                means_of[gp] = stage1(gp)
                if gp >= 2:
                    mlp_back(gp - 2, diff_of.pop(gp - 2))
                if gp >= 1:
                    diff_of[gp - 1] = mlp_front(gp - 1, means_of.pop(gp - 1))
            mlp_back(ngp - 2, diff_of.pop(ngp - 2))
            diff_of[ngp - 1] = mlp_front(ngp - 1, means_of.pop(ngp - 1))
            mlp_back(ngp - 1, diff_of.pop(ngp - 1), last=True)

    # TRN2 allows at most 1 sync wait per instruction (2 on event semaphores).
    # Tile can emit more; split them the same way Bacc.compile() does.
    import bass_rust as _bass_rust
    _bass_rust.move_matmul_waits_to_ldweights(nc.m)
    _bass_rust.generate_event_semaphores(nc)
    return nc


def host_prep(output_ids, features, W1, b1, Wm, bm, W2, b2, rpc=RPC, tch=TCH):
    """Build per-core input maps. Features are cast to bf16 and kept in natural
    [rows, S, H] layout (token s -> partition s//8, chunk s%8 on device); the
    tiny one-hot/weight tensors are repacked for device layout."""
    ids = np.asarray(output_ids)
    nrows = ids.shape[0]
    ncores = nrows // rpc
    feats = np.asarray(features, dtype=np.float32).astype(NPBF16)

    is_sep = ids == SEP_ID
    seg = np.cumsum(is_sep.astype(np.int64), axis=1)
    valid = ~is_sep
    oh = ((seg[:, :, None] == np.arange(NSEG)[None, None, :]) & valid[:, :, None])
    oh = oh.astype(np.float32)                        # [B, S, 8]
    counts = oh.sum(axis=1)                           # [B, 8]
    oh *= (1.0 / np.maximum(counts, 1.0))[:, None, :]  # fold mean scale

    # striped block-diagonal E4: pair p = r4//2 (column block), partition
    # (r4%2)*32+n (n<8; gaps zero), column (g2, b2, i, j) when r4 == g2*2+b2
    eye = np.eye(NSEG, dtype=np.float32)
    base = eye[:, :, None] - eye[:, None, :]          # [n, i, j]
    e4 = np.zeros((2, 128, 2, 2, NSEG, NSEG), np.float32)
    for r4 in range(4):
        off = (r4 % 2) * 32
        e4[r4 // 2, off:off + NSEG, r4 // 2, r4 % 2, :, :] = base
    e4 = np.ascontiguousarray(
        e4.reshape(2, 128, 256).transpose(1, 0, 2).reshape(128, 512)
    ).astype(NPBF16)

    W1 = np.asarray(W1, np.float32)
    Wm = np.asarray(Wm, np.float32)
    W2 = np.asarray(W2, np.float32)
    b1 = np.asarray(b1, np.float32)
    bm = np.asarray(bm, np.float32)
    b2 = np.asarray(b2, np.float32)

    w1p = np.ascontiguousarray(
        W1.reshape(HC, 128, C).transpose(1, 0, 2).reshape(128, HC * C)
    ).astype(NPBF16)
    wm0 = np.ascontiguousarray(Wm[:128]).astype(NPBF16)
    wm1 = np.ascontiguousarray(Wm[128:]).astype(NPBF16)
    w20 = np.ascontiguousarray(W2[:128]).astype(NPBF16)
    w21 = np.ascontiguousarray(W2[128:]).astype(NPBF16)
    b2bc = np.ascontiguousarray(np.broadcast_to(b2[None, :], (128, C)),
                                dtype=np.float32)
    b1c0 = np.ascontiguousarray(b1[:128, None])
    b1c1 = np.ascontiguousarray(b1[128:, None])
    bm0 = np.ascontiguousarray(bm[:128, None])
    bm1 = np.ascontiguousarray(bm[128:, None])

    shared = dict(w1p=w1p, wm0=wm0, wm1=wm1, w20=w20, w21=w21,
                  b1c0=b1c0, b1c1=b1c1, bm0=bm0, bm1=bm1, b2bc=b2bc, e4=e4)

    in_maps = []
    for c in range(ncores):
        rows = slice(c * rpc, (c + 1) * rpc)
        # token s -> (partition p=s//8, chunk t=s%8); col (r, t, n)
        ohT = np.ascontiguousarray(
            oh[rows].reshape(rpc, 128, tch, NSEG)
            .transpose(1, 0, 2, 3).reshape(128, rpc * tch * NSEG)
        ).astype(NPBF16)
        in_maps.append(dict(
            features=np.ascontiguousarray(feats[rows]),
            ohT=ohT, **shared))
    return in_maps


def gather_output(core_outs, rpc=RPC):
    """[ngp*256, C] per core -> [8, 8, B, C]."""
    ncores = len(core_outs)
    ngp = rpc // 4
    full = np.empty((NSEG, NSEG, ncores * rpc, C), np.float32)
    for c, o in enumerate(core_outs):
        o = o.reshape(ngp, 2, 2, NSEG, NSEG, C)       # gp, g2, b2, i, j, c
        o = o.transpose(3, 4, 0, 1, 2, 5).reshape(NSEG, NSEG, rpc, C)
        full[:, :, c * rpc:(c + 1) * rpc, :] = o
    return full


_NC_CACHE = {}


def _get_program():
    key = (RPC, TCH)
    if key not in _NC_CACHE:
        _NC_CACHE[key] = build_program()
    return _NC_CACHE[key]


def run(inputs, trace=False, trace_cores=None):
    nc = _get_program()
    in_maps = host_prep(**inputs)
    res = run_bass_kernel_spmd(
        nc, in_maps, core_ids=list(range(NCORES)),
        trace=trace, trace_cores=trace_cores,
    )
    out = gather_output([r["out"] for r in res.results])
    return out, res


def kernel(**inputs):
    out, _ = run(inputs, trace=False)
    return out



# revision 10
# speedup vs baseline: 2.5706x; 2.5706x over previous
"""Trainium2 Bass kernel: segment-mean + pairwise-diff edge MLP (restructured).

Reference (per batch row b):
  seg = cumsum(ids == 3); valid = ~sep
  means[n] = mean of features[s] over tokens with seg==n & valid (n < 8)
  diff[i,j] = means[i] - means[j]                              # [8,8,H]
  out[i,j]  = relu(relu(diff @ W1 + b1) @ Wm + bm) @ W2 + b2   # [8,8,150]

Key observations driving this version (vs the v1 full-stream kernel):
  1. Only tokens BEFORE the 8th separator contribute (seg < 8). With
     P(sep)=1/8 that is ~60 of 1024 tokens per row -> ~94% of the feature
     HBM traffic in v1 was multiplied by an all-zero one-hot. The host
     packs exactly the contributing tokens (128-token chunks, zero-pad
     tail) -> ~1.6 MB instead of 25 MB per core.
  2. The segment-mean AND the pairwise diff fold into one host-built
     matrix: ohE4[t,(r,i,j)] = oh[t,i]/c_i - oh[t,j]/c_j, so a single
     accumulating matmul per (chunk, h-slice) produces diffT directly in
     PSUM (feats chunk is the stationary operand):
         diffT[h,(r,i,j)] = sum_t feats[t,h] * ohE4[t,(r,i,j)]
     No means stage, no eviction of means, no transpose stage.
  3. Antisymmetry: diff[j,i] = -diff[i,j] and relu breaks it only AFTER
     mm1's product: y = W1^T diffT computed for i<j only (28 of 64
     pairs); h1+ = relu(+y+b1), h1- = relu(-y+b1) reuse the one product.
     mm1/diffT stream width drops 64->28 per row pair. The diagonal
     out[i,i] = f(0) is input-independent -> computed on host in fp32.

Distribution: 128 batch rows sorted by token count, snake-dealt into 16
bins of 8 rows (8 cores x 2 super-groups); per SG tokens are packed
densely into ceil(T/128) chunks of [128 tok, 768] bf16.

Device program per core (2 super-groups, SG = 8 rows = 224 diffT cols):
  diffT: for hc in 6: for chunk: matmul(dp[hc] (+)= featsT_chunk[hc] @
         ohE4_chunk), N=224 moving, feats stationary (FWL bf16).
         hc-major so each 2-hc PSUM bank is cast (fp32->bf16) while later
         hc still accumulate; casts split vector/scalar/gpsimd.
  mm1:   y[ci] = W1^T @ diffT (accumulate 6 h-chunks), ci = c-split
         128+22; h1 = [relu(y+b1) | relu(-y+b1)] -> [csz, 448] bf16.
  mm2:   h2 = relu(Wm^T h1 + bm), k-split 128+22, N=448.
  mm3:   out = h2^T-slices @ W2 + b2 -> 4 x [112, 150] fp32, DMA out.
  PE p-state warmup: dummy matmuls spin the array during the initial
  DMA window (the PE runs its first ~3.4us of activity at 1.2 GHz).

PSUM banks: dp 2 + h1 2 + h2 1+1 + out/warm 2 = 8.
"""

import sys

import numpy as np
import ml_dtypes

if "/opt/trn_rl_repo" not in sys.path:
    sys.path.insert(0, "/opt/trn_rl_repo")

import concourse.bass as bass
import concourse.mybir as mybir
from concourse.bass import ds
from concourse.bass_utils import run_bass_kernel_spmd
from concourse.tile import TileContext

B, S, H, C = 128, 1024, 768, 150
NSEG = 8
SEP_ID = 3
NCORES = 8
NSG = 2                      # super-groups per core
NU = NSEG * (NSEG - 1) // 2  # 28 (i<j) pairs
RSG = B // (NCORES * NSG)    # 8 rows per super-group
NU8 = RSG * NU               # 224 diffT columns per SG
HC = H // 128                # 6 h-chunks
CC = ((0, 128), (128, 22))   # c-dim (150) split
NWARM = 16

F32 = mybir.dt.float32
BF16 = mybir.dt.bfloat16
NPBF16 = ml_dtypes.bfloat16

UI = np.array([i for i in range(NSEG) for j in range(i + 1, NSEG)])
UJ = np.array([j for i in range(NSEG) for j in range(i + 1, NSEG)])


def build_program(nc0, nc1):
    NCT = nc0 + nc1
    nc = bass.Bass("TRN2", target_bir_lowering=False, debug=False)

    feats_d = nc.dram_tensor("feats", [NCT, 128, H], BF16,
                             kind="ExternalInput").ap()
    ohT_d = nc.dram_tensor("ohT", [128, NCT * NU8], BF16,
                           kind="ExternalInput").ap()
    w1p_d = nc.dram_tensor("w1p", [128, HC * C], BF16, kind="ExternalInput").ap()
    wm0_d = nc.dram_tensor("wm0", [128, C], BF16, kind="ExternalInput").ap()
    wm1_d = nc.dram_tensor("wm1", [22, C], BF16, kind="ExternalInput").ap()
    w20_d = nc.dram_tensor("w20", [128, C], BF16, kind="ExternalInput").ap()
    # w21e: rows 0-21 = W2[128:], row 22 = b2 (the matching h2 row is a
    # constant 1 -> mm3 adds b2 inside the matmul, evictions become copies)
    w21e_d = nc.dram_tensor("w21e", [23, C], BF16, kind="ExternalInput").ap()
    b1c0_d = nc.dram_tensor("b1c0", [128, 1], F32, kind="ExternalInput").ap()
    b1c1_d = nc.dram_tensor("b1c1", [22, 1], F32, kind="ExternalInput").ap()
    bm0_d = nc.dram_tensor("bm0", [128, 1], F32, kind="ExternalInput").ap()
    bm1_d = nc.dram_tensor("bm1", [22, 1], F32, kind="ExternalInput").ap()
    out_d = nc.dram_tensor("out", [NSG * 2 * NU8, C], F32,
                           kind="ExternalOutput").ap()

    RELU = mybir.ActivationFunctionType.Relu
    COPY = mybir.ActivationFunctionType.Copy
    ADD = mybir.AluOpType.add
    MAX = mybir.AluOpType.max

    with TileContext(nc) as tc:
        with (
            tc.tile_pool(name="const", bufs=1) as constp,
            tc.tile_pool(name="feat", bufs=1) as featp,
            tc.tile_pool(name="diff", bufs=2) as diffp,
            tc.tile_pool(name="act", bufs=2) as actp,
            tc.tile_pool(name="osb", bufs=2) as osbp,
            tc.tile_pool(name="dps", bufs=2, space="PSUM") as dpsum,
            tc.tile_pool(name="h1ps", bufs=2, space="PSUM") as h1ps,
            tc.tile_pool(name="h2ps0", bufs=1, space="PSUM") as h2ps0,
            tc.tile_pool(name="h2ps1", bufs=1, space="PSUM") as h2ps1,
            tc.tile_pool(name="ops", bufs=2, space="PSUM") as opps,
        ):
            # sync queue: the two per-SG feature streams
            fsb = []
            for sg, (base, n) in enumerate(((0, nc0), (nc0, nc1))):
                t = featp.tile([128, n, H], BF16, tag=f"f{sg}")
                nc.sync.dma_start(
                    out=t, in_=feats_d[ds(base, n)].rearrange("c p h -> p c h"))
                fsb.append(t)
            # scalar queue: fused one-hot first (needed first), then weights
            ohT_sb = constp.tile([128, NCT * NU8], BF16, tag="ohT")
            nc.scalar.dma_start(out=ohT_sb, in_=ohT_d)
            w1_sb = constp.tile([128, HC * C], BF16, tag="w1")
            nc.scalar.dma_start(out=w1_sb, in_=w1p_d)
            wm0_sb = constp.tile([128, C], BF16, tag="wm0")
            nc.scalar.dma_start(out=wm0_sb, in_=wm0_d)
            wm1_sb = constp.tile([22, C], BF16, tag="wm1")
            nc.scalar.dma_start(out=wm1_sb, in_=wm1_d)
            w20_sb = constp.tile([128, C], BF16, tag="w20")
            nc.scalar.dma_start(out=w20_sb, in_=w20_d)
            w21e_sb = constp.tile([23, C], BF16, tag="w21e")
            nc.scalar.dma_start(out=w21e_sb, in_=w21e_d)
            b1_sb, bm_sb = [], []
            for ci, (coff, csz) in enumerate(CC):
                t = constp.tile([csz, 1], F32, tag=f"b1_{ci}")
                nc.scalar.dma_start(out=t, in_=(b1c0_d, b1c1_d)[ci])
                b1_sb.append(t)
                t = constp.tile([csz, 1], F32, tag=f"bm_{ci}")
                nc.scalar.dma_start(out=t, in_=(bm0_d, bm1_d)[ci])
                bm_sb.append(t)

            # PE p-state warmup during the feature-DMA window
            dmy = constp.tile([128, 128], BF16, tag="dmy")
            nc.vector.memset(dmy, 0.0)
            wts = [opps.tile([128, 2, C], F32, tag="op", name=f"warm{i}")
                   for i in range(2)]
            for i in range(NWARM):
                nc.tensor.matmul(wts[i % 2][:, 0, ds(0, 128)], dmy, dmy,
                                 start=True, stop=True)

            def diffT_stage(sg):
                """dp[hc][h, (r8,u)] = sum_tok feats[tok, h]*ohE4[tok, col];
                feats chunk h-slice stationary, ohE4 moving (N=224).
                hc-major so each 2-hc bank is evicted while later hc still
                run; the 3 casts rotate vector/scalar/gpsimd."""
                f = fsb[sg]
                n = (nc0, nc1)[sg]
                base = (0, nc0)[sg]
                diff = diffp.tile([128, HC, NU8], BF16, tag="diff")
                for hp in range(3):
                    dp = dpsum.tile([128, 2, NU8], F32, tag="dp")
                    for k in range(2):
                        hc = 2 * hp + k
                        for c in range(n):
                            nc.tensor.matmul(
                                dp[:, k, :],
                                f[:, c, ds(hc * 128, 128)],
                                ohT_sb[:, ds((base + c) * NU8, NU8)],
                                start=(c == 0), stop=(c == n - 1))
                    dst = diff[:, ds(2 * hp, 2), :]
                    # gpsimd cannot read PSUM; rotate vector/scalar
                    if hp == 1:
                        nc.scalar.activation(dst, dp, COPY)
                    else:
                        nc.vector.tensor_copy(dst, dp)
                return diff

            def mm1(sg, diff):
                """y = W1^T diffT (accumulate over hc); h1 = [relu(y+b1),
                relu(-y+b1)] (the +/- trick: one product, both pair
                orders). ci0 -> vector, minus branch -> scalar."""
                hp = h1ps.tile([128, 2 * NU8], F32, tag="h1p")
                h1 = []
                for ci, (coff, csz) in enumerate(CC):
                    out_ap = hp[ds(0, csz), ds(ci * NU8, NU8)]
                    for hc in range(HC):
                        nc.tensor.matmul(
                            out_ap,
                            w1_sb[:, ds(hc * C + coff, csz)],
                            diff[:, hc, :],
                            start=(hc == 0), stop=(hc == HC - 1))
                for ci, (coff, csz) in enumerate(CC):
                    src = hp[ds(0, csz), ds(ci * NU8, NU8)]
                    hs = actp.tile([csz, 2 * NU8], BF16, tag=f"h1_{ci}")
                    nc.vector.tensor_scalar(hs[:, ds(0, NU8)], src,
                                            b1_sb[ci], 0.0, ADD, MAX)
                    nc.scalar.activation(hs[:, ds(NU8, NU8)], src, RELU,
                                         bias=b1_sb[ci], scale=-1.0)
                    h1.append(hs)
                return h1

            def mm2(sg, h1):
                h2 = []
                for ci, (coff, csz) in enumerate(CC):
                    hp2 = (h2ps0, h2ps1)[ci].tile([csz, 2 * NU8], F32,
                                                  tag=f"h2p{ci}")
                    nc.tensor.matmul(hp2, wm0_sb[:, ds(coff, csz)], h1[0],
                                     start=True, stop=False)
                    nc.tensor.matmul(hp2, wm1_sb[:, ds(coff, csz)], h1[1],
                                     start=False, stop=True)
                    hs = actp.tile([csz + (1 if ci == 1 else 0), 2 * NU8],
                                   BF16, tag=f"h2_{ci}")
                    if ci == 0:
                        nc.scalar.activation(hs, hp2, RELU, bias=bm_sb[0])
                    else:
                        # row 22 stays 1.0 to pair with w21e's b2 row in
                        # mm3; partition-22 start is not a legal AP, so
                        # memset the whole tile then overwrite rows 0-21
                        nc.vector.memset(hs, 1.0)
                        nc.vector.tensor_scalar(hs[ds(0, 22), :], hp2,
                                                bm_sb[1], 0.0, ADD, MAX)
                    h2.append(hs)
                return h2

            def mm3(sg, h2, last=False):
                for t in range(2):
                    op = opps.tile([128, 2, C], F32, tag="op")
                    osb = osbp.tile([112, 2, C], F32, tag="osb")
                    for sl in range(2):
                        s = t * 2 + sl
                        nc.tensor.matmul(op[ds(0, 112), sl, :],
                                         h2[0][:, ds(s * 112, 112)],
                                         w20_sb, start=True, stop=False)
                        nc.tensor.matmul(op[ds(0, 112), sl, :],
                                         h2[1][:, ds(s * 112, 112)],
                                         w21e_sb, start=False, stop=True)
                        if t == 0:
                            nc.scalar.activation(osb[:, sl, :],
                                                 op[ds(0, 112), sl, :], COPY)
                        else:
                            nc.vector.tensor_copy(osb[:, sl, :],
                                                  op[ds(0, 112), sl, :])
                    deng = nc.scalar if (last and t == 1) else nc.sync
                    deng.dma_start(
                        out=out_d[ds(sg * 2 * NU8 + t * 224, 224)].rearrange(
                            "(a p) c -> p a c", a=2),
                        in_=osb)

            # 2-deep software pipeline: SG1's diffT fills the PE while
            # SG0's casts/activations run on vector/scalar/gpsimd.
            d0 = diffT_stage(0)
            d1 = diffT_stage(1)
            h1_0 = mm1(0, d0)
            h1_1 = mm1(1, d1)
            h2_0 = mm2(0, h1_0)
            h2_1 = mm2(1, h1_1)
            mm3(0, h2_0)
            mm3(1, h2_1, last=True)

    # TRN2 allows at most 1 sync wait per instruction (2 on event
    # semaphores); split the tile-emitted multi-waits like Bacc.compile().
    import bass_rust as _bass_rust
    _bass_rust.move_matmul_waits_to_ldweights(nc.m)
    _bass_rust.generate_event_semaphores(nc)
    return nc


def host_prep(output_ids, features, W1, b1, Wm, bm, W2, b2):
    ids = np.asarray(output_ids)
    B_, S_ = ids.shape
    feats = np.asarray(features)
    is_sep = ids == SEP_ID
    seg = np.cumsum(is_sep.astype(np.int64), axis=1)
    valid = (~is_sep) & (seg < NSEG)
    counts = np.stack([((seg == n) & valid).sum(1) for n in range(NSEG)],
                      axis=1).astype(np.float32)
    inv_c = (1.0 / np.maximum(counts, 1.0)).astype(np.float32)
    ntok = valid.sum(1)

    # per-row [8, 28] template: token in segment s contributes row s
    tmpl = np.zeros((B_, NSEG, NU), np.float32)
    for u in range(NU):
        tmpl[:, UI[u], u] += inv_c[:, UI[u]]
        tmpl[:, UJ[u], u] -= inv_c[:, UJ[u]]

    # snake-deal rows (sorted by token count) into 16 bins of 8
    nbins = NCORES * NSG
    rsg = B_ // nbins
    order = np.argsort(-ntok, kind="stable")
    bins = [[] for _ in range(nbins)]
    for rnd in range(rsg):
        chunk = order[rnd * nbins:(rnd + 1) * nbins]
        tgt = range(nbins) if rnd % 2 == 0 else range(nbins - 1, -1, -1)
        for t, bb in zip(tgt, chunk):
            bins[t].append(int(bb))
    Tbin = [int(sum(ntok[bb] for bb in bins[k])) for k in range(nbins)]
    ncs = [max(1, -(-Tbin[k] // 128)) for k in range(nbins)]
    NC = [max(ncs[sg * NCORES:(sg + 1) * NCORES]) for sg in range(NSG)]
    nc0, nc1 = NC
    NCT = nc0 + nc1

    W1 = np.asarray(W1, np.float32)
    Wm = np.asarray(Wm, np.float32)
    W2 = np.asarray(W2, np.float32)
    b1 = np.asarray(b1, np.float32)
    bm = np.asarray(bm, np.float32)
    b2 = np.asarray(b2, np.float32)

    w1p = np.ascontiguousarray(
        W1.reshape(HC, 128, C).transpose(1, 0, 2).reshape(128, HC * C)
    ).astype(NPBF16)
    shared = dict(
        w1p=w1p,
        wm0=np.ascontiguousarray(Wm[:128]).astype(NPBF16),
        wm1=np.ascontiguousarray(Wm[128:]).astype(NPBF16),
        w20=np.ascontiguousarray(W2[:128]).astype(NPBF16),
        w21e=np.ascontiguousarray(
            np.concatenate([W2[128:], b2[None, :]], axis=0)).astype(NPBF16),
        b1c0=np.ascontiguousarray(b1[:128, None]),
        b1c1=np.ascontiguousarray(b1[128:, None]),
        bm0=np.ascontiguousarray(bm[:128, None]),
        bm1=np.ascontiguousarray(bm[128:, None]),
    )

    # diagonal f(0) is input-independent: exact fp32 on host
    y0 = np.maximum(b1, 0.0)
    y1 = np.maximum(y0 @ Wm + bm, 0.0)
    diag = (y1 @ W2 + b2).astype(np.float32)

    in_maps, gather_maps = [], []
    for core in range(NCORES):
        fp = np.zeros((NCT, 128, H), NPBF16)
        ohe = np.zeros((NCT, 128, NU8), np.float32)
        for sg in range(NSG):
            base = 0 if sg == 0 else nc0
            rows = bins[sg * NCORES + core]
            pos = 0
            for r8, bb in enumerate(rows):
                toks = np.nonzero(valid[bb])[0]
                n = len(toks)
                if n == 0:
                    continue
                sl = np.arange(pos, pos + n)
                ch = base + sl // 128
                pt = sl % 128
                fp[ch, pt, :] = feats[bb, toks, :].astype(NPBF16)
                ohe[ch, pt, r8 * NU:(r8 + 1) * NU] = tmpl[bb, seg[bb, toks], :]
                pos += n
        ohT = np.ascontiguousarray(
            ohe.transpose(1, 0, 2).reshape(128, NCT * NU8)).astype(NPBF16)
        in_maps.append(dict(feats=np.ascontiguousarray(fp), ohT=ohT, **shared))

        di, dj, db = [], [], []
        for sg in range(NSG):
            rows = bins[sg * NCORES + core]
            for half in range(2):
                for r8 in range(rsg):
                    bb = rows[r8]
                    for u in range(NU):
                        i, j = (UI[u], UJ[u]) if half == 0 else (UJ[u], UI[u])
                        di.append(i)
                        dj.append(j)
                        db.append(bb)
        gather_maps.append((np.array(di), np.array(dj), np.array(db)))

    aux = dict(nc0=nc0, nc1=nc1, gather_maps=gather_maps, diag=diag, B=B_)
    return in_maps, aux


def gather_output(core_outs, aux):
    full = np.empty((NSEG, NSEG, aux["B"], C), np.float32)
    for i in range(NSEG):
        full[i, i, :, :] = aux["diag"][None, :]
    for core, o in enumerate(core_outs):
        di, dj, db = aux["gather_maps"][core]
        full[di, dj, db, :] = o
    return full


_NC_CACHE = {}


def run(inputs, trace=False, trace_cores=None):
    in_maps, aux = host_prep(**inputs)
    key = (aux["nc0"], aux["nc1"])
    if key not in _NC_CACHE:
        _NC_CACHE[key] = build_program(*key)
    nc = _NC_CACHE[key]
    res = run_bass_kernel_spmd(
        nc, in_maps, core_ids=list(range(NCORES)),
        trace=trace, trace_cores=trace_cores,
    )
    out = gather_output([r["out"] for r in res.results], aux)
    return out, res


def kernel(**inputs):
    out, _ = run(inputs, trace=False)
    return out


# revision 15
# speedup vs baseline: 2.9718x; 1.1561x over previous
"""Trainium2 Bass kernel: segment-mean + pairwise-diff edge MLP (restructured).

Reference (per batch row b):
  seg = cumsum(ids == 3); valid = ~sep
  means[n] = mean of features[s] over tokens with seg==n & valid (n < 8)
  diff[i,j] = means[i] - means[j]                              # [8,8,H]
  out[i,j]  = relu(relu(diff @ W1 + b1) @ Wm + bm) @ W2 + b2   # [8,8,150]

Key observations driving this version (vs the v1 full-stream kernel):
  1. Only tokens BEFORE the 8th separator contribute (seg < 8). With
     P(sep)=1/8 that is ~60 of 1024 tokens per row -> ~94% of the feature
     HBM traffic in v1 was multiplied by an all-zero one-hot. The host
     packs exactly the contributing tokens (128-token chunks, zero-pad
     tail) -> ~1.6 MB instead of 25 MB per core.
  2. The segment-mean AND the pairwise diff fold into one host-built
     matrix: ohE4[t,(r,i,j)] = oh[t,i]/c_i - oh[t,j]/c_j, so a single
     accumulating matmul per (chunk, h-slice) produces diffT directly in
     PSUM (feats chunk is the stationary operand):
         diffT[h,(r,i,j)] = sum_t feats[t,h] * ohE4[t,(r,i,j)]
     No means stage, no eviction of means, no transpose stage.
  3. Antisymmetry: diff[j,i] = -diff[i,j] and relu breaks it only AFTER
     mm1's product: y = W1^T diffT computed for i<j only (28 of 64
     pairs); h1+ = relu(+y+b1), h1- = relu(-y+b1) reuse the one product.
     mm1/diffT stream width drops 64->28 per row pair. The diagonal
     out[i,i] = f(0) is input-independent -> computed on host in fp32.

Distribution: 128 batch rows sorted by token count, snake-dealt into 16
bins of 8 rows (8 cores x 2 super-groups); per SG tokens are packed
densely into ceil(T/128) chunks of [128 tok, 768] bf16.

Device program per core (2 super-groups, SG = 8 rows = 224 diffT cols):
  diffT: for hc in 6: for chunk: matmul(dp[hc] (+)= featsT_chunk[hc] @
         ohE4_chunk), N=224 moving, feats stationary (FWL bf16).
         hc-major so each 2-hc PSUM bank is cast (fp32->bf16) while later
         hc still accumulate; casts split vector/scalar/gpsimd.
  mm1:   y[ci] = W1^T @ diffT (accumulate 6 h-chunks), ci = c-split
         128+22; h1 = [relu(y+b1) | relu(-y+b1)] -> [csz, 448] bf16.
  mm2:   h2 = relu(Wm^T h1 + bm), k-split 128+22, N=448.
  mm3:   out = h2^T-slices @ W2 + b2 -> 4 x [112, 150] fp32, DMA out.
  PE p-state warmup: dummy matmuls spin the array during the initial
  DMA window (the PE runs its first ~3.4us of activity at 1.2 GHz).

PSUM banks: dp 2 + h1 2 + h2 1+1 + out/warm 2 = 8.
"""

import sys

import numpy as np
import ml_dtypes

if "/opt/trn_rl_repo" not in sys.path:
    sys.path.insert(0, "/opt/trn_rl_repo")

import concourse.bass as bass
import concourse.mybir as mybir
from concourse.bass import ds
from concourse.bass_utils import run_bass_kernel_spmd
from concourse.tile import TileContext

B, S, H, C = 128, 1024, 768, 150
NSEG = 8
SEP_ID = 3
NCORES = 8
NSG = 2                      # super-groups per core
NU = NSEG * (NSEG - 1) // 2  # 28 (i<j) pairs
RSG = B // (NCORES * NSG)    # 8 rows per super-group
NU8 = RSG * NU               # 224 diffT columns per SG
HC = H // 128                # 6 h-chunks
CC = ((0, 128), (128, 22))   # c-dim (150) split
NWARM = 26
# packed bf16 weight tensor column offsets
WPK_W1 = 0            # [128, HC*C] w1 h-major
WPK_WM0 = HC * C      # [128, C]
WPK_W20 = HC * C + C  # [128, C]
WPK_WM1 = HC * C + 2 * C   # [22, C]
WPK_W21E = HC * C + 3 * C  # [23, C] (row 22 = b2)
WPK_COLS = HC * C + 4 * C

F32 = mybir.dt.float32
BF16 = mybir.dt.bfloat16
NPBF16 = ml_dtypes.bfloat16

UI = np.array([i for i in range(NSEG) for j in range(i + 1, NSEG)])
UJ = np.array([j for i in range(NSEG) for j in range(i + 1, NSEG)])


def build_program(nc0, nc1):
    NCT = nc0 + nc1
    nc = bass.Bass("TRN2", target_bir_lowering=False, debug=False)

    feats_d = nc.dram_tensor("feats", [NCT, 128, H], BF16,
                             kind="ExternalInput").ap()
    ohT_d = nc.dram_tensor("ohT", [128, NCT * NU8], BF16,
                           kind="ExternalInput").ap()
    # single packed bf16 weight tensor (one DMA): w1 | wm0 | w20 | wm1 | w21e
    # (w21e rows 0-21 = W2[128:], row 22 = b2: the matching h2 row is a
    # constant 1 -> mm3 adds b2 inside the matmul, evictions become copies)
    wpk_d = nc.dram_tensor("wpk", [128, WPK_COLS], BF16,
                           kind="ExternalInput").ap()
    # packed fp32 biases: col0 = b1[:128], col1 = bm[:128],
    # col2 rows0-21 = b1[128:], col3 rows0-21 = bm[128:]
    bias_d = nc.dram_tensor("biasp", [128, 4], F32, kind="ExternalInput").ap()
    out_d = nc.dram_tensor("out", [NSG * 2 * NU8, C], F32,
                           kind="ExternalOutput").ap()

    RELU = mybir.ActivationFunctionType.Relu
    COPY = mybir.ActivationFunctionType.Copy
    ADD = mybir.AluOpType.add
    MAX = mybir.AluOpType.max

    with TileContext(nc) as tc:
        with (
            tc.tile_pool(name="const", bufs=1) as constp,
            tc.tile_pool(name="feat", bufs=1) as featp,
            tc.tile_pool(name="diff", bufs=2) as diffp,
            tc.tile_pool(name="act", bufs=2) as actp,
            tc.tile_pool(name="osb", bufs=2) as osbp,
            tc.tile_pool(name="dps", bufs=2, space="PSUM") as dpsum,
            tc.tile_pool(name="h1ps", bufs=2, space="PSUM") as h1ps,
            tc.tile_pool(name="h2ps0", bufs=1, space="PSUM") as h2ps0,
            tc.tile_pool(name="h2ps1", bufs=1, space="PSUM") as h2ps1,
            tc.tile_pool(name="ops", bufs=2, space="PSUM") as opps,
        ):
            # sync queue: the two per-SG feature streams
            fsb = []
            for sg, (base, n) in enumerate(((0, nc0), (nc0, nc1))):
                t = featp.tile([128, n, H], BF16, tag=f"f{sg}")
                nc.sync.dma_start(
                    out=t, in_=feats_d[ds(base, n)].rearrange("c p h -> p c h"))
                fsb.append(t)
            # scalar queue: fused one-hot first (needed first), then weights
            ohT_sb = constp.tile([128, NCT * NU8], BF16, tag="ohT")
            nc.scalar.dma_start(out=ohT_sb, in_=ohT_d)
            wpk_sb = constp.tile([128, WPK_COLS], BF16, tag="wpk")
            nc.scalar.dma_start(out=wpk_sb, in_=wpk_d)
            bias_sb = constp.tile([128, 4], F32, tag="biasp")
            nc.scalar.dma_start(out=bias_sb, in_=bias_d)
            w1_sb = wpk_sb[:, ds(WPK_W1, HC * C)]
            wm0_sb = wpk_sb[:, ds(WPK_WM0, C)]
            w20_sb = wpk_sb[:, ds(WPK_W20, C)]
            wm1_sb = wpk_sb[ds(0, 22), ds(WPK_WM1, C)]
            w21e_sb = wpk_sb[ds(0, 23), ds(WPK_W21E, C)]
            b1_sb = [bias_sb[:, ds(0, 1)], bias_sb[ds(0, 22), ds(2, 1)]]
            bm_sb = [bias_sb[:, ds(1, 1)], bias_sb[ds(0, 22), ds(3, 1)]]

            # PE p-state warmup during the feature-DMA window
            dmy = constp.tile([128, 128], BF16, tag="dmy")
            nc.vector.memset(dmy, 0.0)
            wts = [opps.tile([128, 2, C], F32, tag="op", name=f"warm{i}")
                   for i in range(2)]
            for i in range(NWARM):
                nc.tensor.matmul(wts[i % 2][:, 0, ds(0, 128)], dmy, dmy,
                                 start=True, stop=True)

            def diffT_stage(sg):
                """dp[hc][h, (r8,u)] = sum_tok feats[tok, h]*ohE4[tok, col];
                feats chunk h-slice stationary, ohE4 moving (N=224).
                hc-major so each 2-hc bank is evicted while later hc still
                run; the 3 casts rotate vector/scalar/gpsimd."""
                f = fsb[sg]
                n = (nc0, nc1)[sg]
                base = (0, nc0)[sg]
                diff = diffp.tile([128, HC, NU8], BF16, tag="diff")
                for hp in range(3):
                    dp = dpsum.tile([128, 2, NU8], F32, tag="dp")
                    for k in range(2):
                        hc = 2 * hp + k
                        for c in range(n):
                            nc.tensor.matmul(
                                dp[:, k, :],
                                f[:, c, ds(hc * 128, 128)],
                                ohT_sb[:, ds((base + c) * NU8, NU8)],
                                start=(c == 0), stop=(c == n - 1))
                    dst = diff[:, ds(2 * hp, 2), :]
                    # gpsimd cannot read PSUM; rotate vector/scalar
                    if hp == 1:
                        nc.scalar.activation(dst, dp, COPY)
                    else:
                        nc.vector.tensor_copy(dst, dp)
                return diff

            def mm1(sg, diff):
                """y = W1^T diffT (accumulate over hc); h1 = [relu(y+b1),
                relu(-y+b1)] (the +/- trick: one product, both pair
                orders). ci0 -> vector, minus branch -> scalar."""
                hp = h1ps.tile([128, 2 * NU8], F32, tag="h1p")
                h1 = []
                for ci, (coff, csz) in enumerate(CC):
                    out_ap = hp[ds(0, csz), ds(ci * NU8, NU8)]
                    for hc in range(HC):
                        nc.tensor.matmul(
                            out_ap,
                            w1_sb[:, ds(hc * C + coff, csz)],
                            diff[:, hc, :],
                            start=(hc == 0), stop=(hc == HC - 1))
                for ci, (coff, csz) in enumerate(CC):
                    src = hp[ds(0, csz), ds(ci * NU8, NU8)]
                    hs = actp.tile([csz, 2 * NU8], BF16, tag=f"h1_{ci}")
                    nc.vector.tensor_scalar(hs[:, ds(0, NU8)], src,
                                            b1_sb[ci], 0.0, ADD, MAX)
                    nc.scalar.activation(hs[:, ds(NU8, NU8)], src, RELU,
                                         bias=b1_sb[ci], scale=-1.0)
                    h1.append(hs)
                return h1

            def mm2(sg, h1):
                h2 = []
                for ci, (coff, csz) in enumerate(CC):
                    hp2 = (h2ps0, h2ps1)[ci].tile([csz, 2 * NU8], F32,
                                                  tag=f"h2p{ci}")
                    nc.tensor.matmul(hp2, wm0_sb[:, ds(coff, csz)], h1[0],
                                     start=True, stop=False)
                    nc.tensor.matmul(hp2, wm1_sb[:, ds(coff, csz)], h1[1],
                                     start=False, stop=True)
                    hs = actp.tile([csz + (1 if ci == 1 else 0), 2 * NU8],
                                   BF16, tag=f"h2_{ci}")
                    if ci == 0:
                        nc.scalar.activation(hs, hp2, RELU, bias=bm_sb[0])
                    else:
                        # row 22 stays 1.0 to pair with w21e's b2 row in
                        # mm3; partition-22 start is not a legal AP, so
                        # memset the whole tile then overwrite rows 0-21
                        nc.vector.memset(hs, 1.0)
                        nc.vector.tensor_scalar(hs[ds(0, 22), :], hp2,
                                                bm_sb[1], 0.0, ADD, MAX)
                    h2.append(hs)
                return h2

            def mm3(sg, h2, last=False):
                # evictions all on scalar (free at this phase) so the op
                # PSUM reuse is never queued behind vector work
                osb = osbp.tile([112, 4, C], F32, tag="osb")
                for t in range(2):
                    op = opps.tile([128, 2, C], F32, tag="op")
                    for sl in range(2):
                        s = t * 2 + sl
                        nc.tensor.matmul(op[ds(0, 112), sl, :],
                                         h2[0][:, ds(s * 112, 112)],
                                         w20_sb, start=True, stop=False)
                        nc.tensor.matmul(op[ds(0, 112), sl, :],
                                         h2[1][:, ds(s * 112, 112)],
                                         w21e_sb, start=False, stop=True)
                        nc.scalar.activation(osb[:, s, :],
                                             op[ds(0, 112), sl, :], COPY)
                deng = nc.scalar if last else nc.sync
                deng.dma_start(
                    out=out_d[ds(sg * 2 * NU8, 448)].rearrange(
                        "(a p) c -> p a c", a=4),
                    in_=osb)

            # 2-deep software pipeline: SG1's diffT fills the PE while
            # SG0's casts/activations run on vector/scalar/gpsimd.
            d0 = diffT_stage(0)
            d1 = diffT_stage(1)
            h1_0 = mm1(0, d0)
            h1_1 = mm1(1, d1)
            h2_0 = mm2(0, h1_0)
            h2_1 = mm2(1, h1_1)
            mm3(0, h2_0)
            mm3(1, h2_1, last=True)

    # TRN2 allows at most 1 sync wait per instruction (2 on event
    # semaphores); split the tile-emitted multi-waits like Bacc.compile().
    import bass_rust as _bass_rust
    _bass_rust.move_matmul_waits_to_ldweights(nc.m)
    _bass_rust.generate_event_semaphores(nc)
    return nc


def host_prep(output_ids, features, W1, b1, Wm, bm, W2, b2):
    ids = np.asarray(output_ids)
    B_, S_ = ids.shape
    feats = np.asarray(features)
    is_sep = ids == SEP_ID
    seg = np.cumsum(is_sep.astype(np.int64), axis=1)
    valid = (~is_sep) & (seg < NSEG)
    counts = np.stack([((seg == n) & valid).sum(1) for n in range(NSEG)],
                      axis=1).astype(np.float32)
    inv_c = (1.0 / np.maximum(counts, 1.0)).astype(np.float32)
    ntok = valid.sum(1)

    # per-row [8, 28] template: token in segment s contributes row s
    tmpl = np.zeros((B_, NSEG, NU), np.float32)
    for u in range(NU):
        tmpl[:, UI[u], u] += inv_c[:, UI[u]]
        tmpl[:, UJ[u], u] -= inv_c[:, UJ[u]]

    # snake-deal rows (sorted by token count) into 16 bins of 8
    nbins = NCORES * NSG
    rsg = B_ // nbins
    order = np.argsort(-ntok, kind="stable")
    bins = [[] for _ in range(nbins)]
    for rnd in range(rsg):
        chunk = order[rnd * nbins:(rnd + 1) * nbins]
        tgt = range(nbins) if rnd % 2 == 0 else range(nbins - 1, -1, -1)
        for t, bb in zip(tgt, chunk):
            bins[t].append(int(bb))
    Tbin = [int(sum(ntok[bb] for bb in bins[k])) for k in range(nbins)]
    ncs = [max(1, -(-Tbin[k] // 128)) for k in range(nbins)]
    NC = [max(ncs[sg * NCORES:(sg + 1) * NCORES]) for sg in range(NSG)]
    nc0, nc1 = NC
    NCT = nc0 + nc1

    W1 = np.asarray(W1, np.float32)
    Wm = np.asarray(Wm, np.float32)
    W2 = np.asarray(W2, np.float32)
    b1 = np.asarray(b1, np.float32)
    bm = np.asarray(bm, np.float32)
    b2 = np.asarray(b2, np.float32)

    wpk = np.zeros((128, WPK_COLS), np.float32)
    wpk[:, WPK_W1:WPK_W1 + HC * C] = (
        W1.reshape(HC, 128, C).transpose(1, 0, 2).reshape(128, HC * C))
    wpk[:, WPK_WM0:WPK_WM0 + C] = Wm[:128]
    wpk[:, WPK_W20:WPK_W20 + C] = W2[:128]
    wpk[:22, WPK_WM1:WPK_WM1 + C] = Wm[128:]
    wpk[:22, WPK_W21E:WPK_W21E + C] = W2[128:]
    wpk[22, WPK_W21E:WPK_W21E + C] = b2
    biasp = np.zeros((128, 4), np.float32)
    biasp[:, 0] = b1[:128]
    biasp[:, 1] = bm[:128]
    biasp[:22, 2] = b1[128:]
    biasp[:22, 3] = bm[128:]
    shared = dict(
        wpk=np.ascontiguousarray(wpk.astype(NPBF16)),
        biasp=np.ascontiguousarray(biasp),
    )

    # diagonal f(0) is input-independent: exact fp32 on host
    y0 = np.maximum(b1, 0.0)
    y1 = np.maximum(y0 @ Wm + bm, 0.0)
    diag = (y1 @ W2 + b2).astype(np.float32)

    in_maps, gather_maps = [], []
    for core in range(NCORES):
        fp = np.zeros((NCT, 128, H), NPBF16)
        ohe = np.zeros((NCT, 128, NU8), np.float32)
        for sg in range(NSG):
            base = 0 if sg == 0 else nc0
            rows = bins[sg * NCORES + core]
            pos = 0
            for r8, bb in enumerate(rows):
                toks = np.nonzero(valid[bb])[0]
                n = len(toks)
                if n == 0:
                    continue
                sl = np.arange(pos, pos + n)
                ch = base + sl // 128
                pt = sl % 128
                fp[ch, pt, :] = feats[bb, toks, :].astype(NPBF16)
                ohe[ch, pt, r8 * NU:(r8 + 1) * NU] = tmpl[bb, seg[bb, toks], :]
                pos += n
        ohT = np.ascontiguousarray(
            ohe.transpose(1, 0, 2).reshape(128, NCT * NU8)).astype(NPBF16)
        in_maps.append(dict(feats=np.ascontiguousarray(fp), ohT=ohT, **shared))

        di, dj, db = [], [], []
        for sg in range(NSG):
            rows = bins[sg * NCORES + core]
            for half in range(2):
                for r8 in range(rsg):
                    bb = rows[r8]
                    for u in range(NU):
                        i, j = (UI[u], UJ[u]) if half == 0 else (UJ[u], UI[u])
                        di.append(i)
                        dj.append(j)
                        db.append(bb)
        gather_maps.append((np.array(di), np.array(dj), np.array(db)))

    aux = dict(nc0=nc0, nc1=nc1, gather_maps=gather_maps, diag=diag, B=B_)
    return in_maps, aux


def gather_output(core_outs, aux):
    full = np.empty((NSEG, NSEG, aux["B"], C), np.float32)
    for i in range(NSEG):
        full[i, i, :, :] = aux["diag"][None, :]
    for core, o in enumerate(core_outs):
        di, dj, db = aux["gather_maps"][core]
        full[di, dj, db, :] = o
    return full


_NC_CACHE = {}


def run(inputs, trace=False, trace_cores=None):
    in_maps, aux = host_prep(**inputs)
    key = (aux["nc0"], aux["nc1"])
    if key not in _NC_CACHE:
        _NC_CACHE[key] = build_program(*key)
    nc = _NC_CACHE[key]
    res = run_bass_kernel_spmd(
        nc, in_maps, core_ids=list(range(NCORES)),
        trace=trace, trace_cores=trace_cores,
    )
    out = gather_output([r["out"] for r in res.results], aux)
    return out, res


def kernel(**inputs):
    out, _ = run(inputs, trace=False)
    return out
